# revision 6
# baseline (speedup 1.0000x reference)
"""Raw-bass Trainium2 kernel for nn_NanEmbedOld, v46 (final).

out[n, d] = mean_f(x[n, f] * W[f, d] + b[f, d]) = x @ (W/F) + sum_f(b/F)

v38 = v34 minus the vestigial ev_sem/es_sem then_inc riders on the two
  final epilogue ops (nothing waits those sems since the store moved to
  t_sem gating in v26; the update riders delayed both cascade heads by
  ~8ns). v34 vs v30: banks retuned [128,416,224,144,112] (was [128,384,256,192,64]).
  The end barrier is a serialized round-robin whose FIRST hop is Scalar's,
  so barrier-done = Scalar-ready + 7 hops; Scalar (ACT-D) was last-ready.
  Shrinking D (ACT's last bank) and growing C (V's last bank, 5 hops of
  slack) rebalances: 9658 -> 9622 mean. B->A2 shift keeps ACT-B's in-order
  completion from binding ACT-D. Curve: D192/C64 9658, D160/C96 9629,
  D144/C112 9622 (min), D128/C128 9627. v30 notes:
  - Output store issued from the idle Pool (GpSimd) engine instead of
    Sync, gated at t_sem>=3: Pool's DMA dispatch is ~25ns seq time with
    async SWDGE descriptor generation, so Sync no longer serializes the
    end-of-program barrier behind a 598ns DGE config + drain. All of
    Sync's work (input DMAs) is pre-window.
  - Last-bank epilogues swapped: V takes C (64 cols, cheapest op) at
    t_sem>=5, ACT takes D (192) at t_sem>=4, so both final psum->SBUF
    ops finish ~15.65-15.70us instead of 15.73/15.74.
Everything else (one input DMA, 10 f32r matmuls into 5 banks, W,b
pre-scaled 1/F on host, strip) = v26.
"""

import numpy as np

N, F, D = 8192, 256, 64
NCORES = 8
ROWS = N // NCORES
KCH = F // 128
HDR = D + 128
COLS = HDR + ROWS
BANKS = [128, 416, 224, 128, 128]  # A1,A2,B,D,C; V: A1,A2,C  ACT: B,D

MM_F32R = True

_NC_CACHE = {}


def _strip_framework_overhead(nc):
    for fn in nc.m.functions:
        for bi, blk in enumerate(fn.blocks):
            name = blk.name or ""
            if not (bi == 0 or name.endswith("_end")):
                continue
            keep = []
            for inst in blk.instructions:
                tname = type(inst).__name__
                if tname in ("InstDrain", "InstEventSemaphore"):
                    continue
                if bi == 0 and tname == "InstMemset" and "const-" in str(inst.outs):
                    continue
                keep.append(inst)
            blk.instructions = keep


def _build_nc():
    import concourse.bass as bass
    import concourse.mybir as mybir

    f32 = mybir.dt.float32
    mm_dt = mybir.dt.float32r
    A1, A2, B, Dk, C = BANKS
    oA1, oA2, oB = 0, A1, A1 + A2
    oD, oC = A1 + A2 + B, A1 + A2 + B + Dk
    Ident = mybir.ActivationFunctionType.Identity
    Copy = mybir.ActivationFunctionType.Copy

    nc = bass.Bass(
        "TRN2",
        target_bir_lowering=False,
        debug=False,
        enable_asserts=False,
        num_devices=NCORES,
    )

    ins = nc.dram_tensor("ins", [128, KCH, COLS], mm_dt, kind="ExternalInput").ap()
    outT = nc.dram_tensor("outT", [D, ROWS], f32, kind="ExternalOutput").ap()

    with (
        nc.semaphore("x_sem") as x_sem,
        nc.semaphore("t_sem") as t_sem,
        nc.semaphore("v_sem") as v_sem,
        nc.semaphore("ev_sem") as ev_sem,
        nc.semaphore("es_sem") as es_sem,
        nc.semaphore("out_sem") as out_sem,
        nc.sbuf_tensor("t_t", [128, KCH, COLS], mm_dt) as t_t,
        nc.sbuf_tensor("bsum_t", [D, 1], f32) as bsum_t,
        nc.sbuf_tensor("scr_t", [2, 2], f32) as scr_t,
        nc.sbuf_tensor("o_t", [D, ROWS], f32) as o_t,
        nc.psum_tensor("pA1", [D, A1], f32) as pA1,
        nc.psum_tensor("pA2", [D, A2], f32) as pA2,
        nc.psum_tensor("pB", [D, B], f32) as pB,
        nc.psum_tensor("pC", [D, C], f32) as pC,
        nc.psum_tensor("pD", [D, Dk], f32) as pD,
        nc.Block() as block,
    ):
        banks = (
            (pA1, oA1, A1), (pA2, oA2, A2), (pB, oB, B), (pD, oD, Dk), (pC, oC, C)
        )

        @block.sync
        def _(sync):
            sync.dma_start(t_t[:], ins[:]).then_inc(x_sem, 16)

        @block.gpsimd
        def _(gpsimd):
            # Pool-issued store: dispatch is cheap and descriptor generation
            # is async SWDGE, so descriptor reads of o_t trail the gate by
            # well over the remaining epilogue writes.
            gpsimd.dma_start(outT[:], o_t[:]).wait_op(
                t_sem, 2, "sem-ge"
            ).then_inc(out_sem, 16)

        @block.tensor
        def _(tensor):
            tensor.wait_ge(x_sem, 16)
            for p, off, ln in banks:
                for k in range(KCH):
                    mm = nc.tensor.matmul(
                        p[:],
                        t_t[:, k, 0:D],
                        t_t[:, k, HDR + off : HDR + off + ln],
                        start=(k == 0),
                        stop=(k == KCH - 1),
                    )
                mm.then_inc(t_sem, 1)

        @block.vector
        def _(vector):
            vector.wait_ge(x_sem, 16)
            nc.vector.reduce_sum(
                bsum_t[:],
                t_t[0:D, :, D:HDR].bitcast(f32),
                axis=mybir.AxisListType.XY,
            )
            vector.drain().then_inc(v_sem, 1)
            nc.vector.tensor_scalar_add(
                o_t[:, oA1 : oA1 + A1], pA1[:], bsum_t[:]
            )._wait_ge(t_sem, 1)
            nc.vector.tensor_scalar_add(
                o_t[:, oA2 : oA2 + A2], pA2[:], bsum_t[:]
            )._wait_ge(t_sem, 2)
            nc.vector.tensor_scalar_add(
                o_t[:, oC : oC + C], pC[:], bsum_t[:]
            )._wait_ge(t_sem, 5)

        @block.scalar
        def _(scalar):
            # dummy ACT: pulls the activation-table load under the matmuls
            scalar.wait_ge(x_sem, 16)
            nc.scalar.activation(
                scr_t[:], t_t[0:2, 0, 0:2].bitcast(f32), Copy, bias=0.0, scale=0.0
            )
            scalar.wait_ge(v_sem, 1)
            nc.scalar.activation(
                o_t[:, oB : oB + B], pB[:], Ident, bias=bsum_t[:], scale=1.0
            )._wait_ge(t_sem, 3)
            nc.scalar.activation(
                o_t[:, oD : oD + Dk], pD[:], Ident, bias=bsum_t[:], scale=1.0
            )._wait_ge(t_sem, 4)

    _strip_framework_overhead(nc)
    return nc


def _get_nc():
    if "nc" not in _NC_CACHE:
        _NC_CACHE["nc"] = _build_nc()
    return _NC_CACHE["nc"]


def _prep_inputs(x, W, b):
    x = np.ascontiguousarray(x, dtype=np.float32)
    W = np.asarray(W, np.float32) * (1.0 / F)
    b = np.asarray(b, np.float32) * (1.0 / F)
    hdr = np.zeros((128, KCH, HDR), np.float32)
    hdr[:, :, 0:D] = W.reshape(KCH, 128, D).transpose(1, 0, 2)
    hdr[0:D, :, D:HDR] = b.T.reshape(D, KCH, 128)
    in_maps = []
    for i in range(NCORES):
        xi = x[i * ROWS : (i + 1) * ROWS]
        img = np.empty((128, KCH, COLS), np.float32)
        img[:, :, 0:HDR] = hdr
        img[:, :, HDR:] = xi.reshape(ROWS, KCH, 128).transpose(2, 1, 0)
        in_maps.append({"ins": img})
    return in_maps


def kernel(x, W, b):
    from concourse.bass_utils import run_bass_kernel_spmd

    in_maps = _prep_inputs(x, W, b)
    nc = _get_nc()
    res = run_bass_kernel_spmd(nc, in_maps, core_ids=list(range(NCORES)))
    return np.concatenate(
        [np.ascontiguousarray(r["outT"]).T for r in res.results], axis=0
    )
